# revision 38
# baseline (speedup 1.0000x reference)
"""Trainium2 Bass kernel for DifferentiableDLT (batched weighted-DLT homography fit).

Contract: kernel(**inputs) takes FULL inputs
    flow (64, 2, 320, 576) f32, mask (64, 1, 320, 576) f32, img_h, img_w
and returns the FULL output (64, 3, 3) f32.

Strategy (pure data parallel, 8 batches/core x 8 cores):
  The 1024 sample points form a fixed separable 32x32 grid, so bilinear
  sampling touches only 64 of 320 rows (32 pairs of adjacent rows) and 64 of
  576 columns.  All row/column indices are compile-time constants, so per
  core we:
    1. Fetch the needed rows with STATIC strided HW DMAs (no gpsimd gather):
       every 4th grid row is a uniform 37-row stride in HBM, so 5 flow DMAs
       (pair-rows of both channels of all 8 batches per s-slot) and 10 mask
       DMAs cover everything.
    2. Select the 64 needed columns with 9 uniform-stride-run copies per
       tensor (DVE), merging both bilinear column taps into each copy.
    3. Bilinear lerp in y then x (DVE tensor ops with constant weight tiles).
    4. One PE transpose-with-scale puts points on partitions; a constant
       grid-offset tile is added during the PSUM drain.
    5. Hartley-normalize dst with a COMPILE-TIME scale sigma0 (any scale
       choice changes the result only through the 1e-6 regularizer) and a
       runtime mean; build weighted features D = [w, w*cx, w*cy, w*r2]; all
       24+ moments come from one wide PE matmul C6^T @ D plus a block-fold.
    6. Assemble the 8x9 augmented normal equations directly per batch
       (batch on partitions) via 4 accumulating PE matmuls against constant
       E-matrices with sigma0/T_src column scaling folded in; solve with
       unpivoted in-place Gauss-Jordan.
    7. Denormalize in the pre-scaled solution space (runtime mean only),
       sign/scale fix, support gate, DMA out (8,3,3).
"""

import dataclasses
import math
import numpy as np

import concourse.bass as bass
import concourse.bacc as bacc
import concourse.mybir as mybir
from concourse import tile
from concourse import bass_utils

F32 = mybir.dt.float32
ALU = mybir.AluOpType

NCORES = 8
BPC = 8          # batches per core
HF, WF = 320, 576
HW = HF * WF
NG = 32          # grid is NG x NG points
NPTS = NG * NG
EPS = 1e-6

# ---------------------------------------------------------------------------
# host-side constant computation
# ---------------------------------------------------------------------------


def _grid_1d(size, n):
    m = int(size * 0.05)
    return np.linspace(m, size - m - 1, n)


def _wrap16(idxlist):
    """ap_gather index wrapping: list pos k -> partition k%16, slot k//16,
    replicated across the 8 gpsimd cores (16-partition groups)."""
    n = len(idxlist)
    base = np.zeros((16, n // 16), np.int16)
    for k, v in enumerate(idxlist):
        base[k % 16, k // 16] = v
    return np.tile(base, (8, 1))


class _Consts:
    def __init__(self, img_h, img_w):
        ys = _grid_1d(HF, NG)
        xs = _grid_1d(WF, NG)
        y0 = np.floor(ys).astype(np.int64)
        x0 = np.floor(xs).astype(np.int64)
        wy = (ys - y0).astype(np.float32)
        wx = (xs - x0).astype(np.float32)
        self.x0 = x0
        self.y0 = y0
        # ap_gather column-select index tables (CL0=28-based, NCW=536 row pitch)
        CL0 = int(x0[0])
        iF = []
        for h in range(2):
            lst = []
            for s in (2 * h, 2 * h + 1):
                for g in range(2):
                    for i in range(NG):
                        for c2 in range(2):
                            lst.append(s * 1072 + g * 536 + int(x0[i]) - CL0 + c2)
            iF.append(_wrap16(lst))
        self.giF = iF
        lstm = []
        for s in range(4):
            for i in range(NG):
                for c2 in range(2):
                    lstm.append(s * 536 + int(x0[i]) - CL0 + c2)
        self.giM = _wrap16(lstm)
        sx = np.float32((img_w - 1) / max(WF - 1, 1))
        sy = np.float32((img_h - 1) / max(HF - 1, 1))
        self.sx, self.sy = float(sx), float(sy)

        # constant Hartley normalization of source points (image coords)
        j = np.arange(NPTS) // NG
        i = np.arange(NPTS) % NG
        gx = xs[i] * np.float64(sx)
        gy = ys[j] * np.float64(sy)
        mx, my = gx.mean(), gy.mean()
        cxs, cys = gx - mx, gy - my
        s_src = max(np.sqrt(cxs * cxs + cys * cys).mean() / math.sqrt(2.0), 1e-8)
        u = cxs / s_src
        v = cys / s_src
        a_ts = 1.0 / s_src
        c_ts = -mx / s_src
        d_ts = -my / s_src
        self.ca = float(np.float32(c_ts / a_ts))
        self.da = float(np.float32(d_ts / a_ts))
        SIG0 = s_src  # compile-time dst normalization scale

        # ---- lerp weight tiles ----
        p = np.arange(128)
        sfree = np.arange(256) // 64
        self.WYF = wy[(p // 16)[:, None] * 4 + sfree[None, :]].astype(np.float32)
        pm = np.arange(64)
        self.WYM = wy[(pm // 8)[:, None] * 4 + sfree[None, :]].astype(np.float32)
        i128 = np.arange(128) % 32
        self.WX2 = np.tile(wx[i128][None, :], (128, 1)).astype(np.float32)

        # ---- transpose-scale diag + grid offsets ----
        # sampF partition p = kd*16 + b*2 + c ; psF free j = same indexing
        jj = np.arange(128)
        c_j = jj % 2
        kd_j = jj // 16
        sxy = np.where(c_j == 0, np.float64(sx), np.float64(sy))
        self.SXYD = (np.eye(128) * sxy[None, :]).astype(np.float32)
        # GRIDC[pl=(s,i)][j=(kd,b,c)]
        pl = np.arange(128)
        s_pl = pl // 32
        i_pl = pl % 32
        gxv = xs * np.float64(sx)
        gyv = ys * np.float64(sy)
        G = np.where(
            c_j[None, :] == 0,
            gxv[i_pl][:, None],
            gyv[(kd_j[None, :] * 4 + s_pl[:, None])],
        )
        self.GRIDC = G.astype(np.float32)

        # ---- point-feature matrix C6 (128, 8*6): F=[1, uu, uv, u, vv, v] ----
        # ("ones" first so the sum-of-weights moment lands on partition 0)
        # device point order n' = t*128 + pl, pl = s*32+i, grid row k = t*4+s
        PERM = [5, 0, 1, 2, 3, 4]  # new m-index -> old feature index
        feats = np.stack([u * u, u * v, u, v * v, v, np.ones_like(u)], -1)[:, PERM]
        tt = np.arange(NPTS) // 128
        plv = np.arange(NPTS) % 128
        orig = (tt * 4 + plv // 32) * 32 + plv % 32
        self.C6 = np.ascontiguousarray(
            feats[orig].reshape(8, 128, 6).transpose(1, 0, 2).reshape(128, 48)
        ).astype(np.float32)

        # ---- E matrices: AUG[b][r*9+c] = sum_q sum_m M6[m,(q,b)] * EQF[m, 72q + r*9+c]
        E = np.zeros((4, 6, 72))
        sym = [[0, 1, 2], [1, 3, 4], [2, 4, 5]]
        for r in range(3):
            for c in range(3):
                m = sym[r][c]
                E[0, m, r * 9 + c] += 1
                E[0, m, (r + 3) * 9 + (c + 3)] += 1
        cr = [[0, 1], [1, 3], [2, 4]]
        for q, r0 in ((1, 0), (2, 3)):
            for r in range(3):
                for c2 in range(2):
                    m = cr[r][c2]
                    E[q, m, (r0 + r) * 9 + 6 + c2] += -1
                    E[q, m, (6 + c2) * 9 + (r0 + r)] += -1
            for r, m in ((0, 2), (1, 4), (2, 5)):
                E[q, m, (r0 + r) * 9 + 8] += 1
        rb = [[0, 1], [1, 3]]
        for r in range(2):
            for c2 in range(2):
                E[3, rb[r][c2], (6 + r) * 9 + 6 + c2] += 1
        E[3, 2, 6 * 9 + 8] += -1
        E[3, 4, 7 * 9 + 8] += -1
        E = E[:, PERM, :]  # match the C6 feature reorder
        # fold sigma0 moment normalization
        inv0 = 1.0 / SIG0
        E[1] *= inv0
        E[2] *= inv0
        E[3] *= inv0 * inv0
        # fold solution-space column scaling (y = S^-1 h)
        Scc = np.ones(9)
        Scc[[0, 1, 3, 4]] = 1.0 / (a_ts * SIG0)
        Scc[[2, 5]] = 1.0 / SIG0
        Scc[[6, 7]] = 1.0 / a_ts
        colidx = np.arange(72) % 9
        E = E * Scc[colidx][None, None, :]
        self.EQF = np.ascontiguousarray(
            E.transpose(1, 0, 2).reshape(6, 288)
        ).astype(np.float32)
        self.epsd = (EPS * Scc[:8]).astype(np.float32)  # diag regularizer values

        self.IDN64 = np.eye(64, dtype=np.float32)


# ---------------------------------------------------------------------------
# device program
# ---------------------------------------------------------------------------


def _dview(ap, offset, dims):
    """Custom strided DRAM view (strides/offset in elements)."""
    return dataclasses.replace(ap, offset=int(offset), ap=[list(d) for d in dims])


def _build_program(cc: _Consts):
    nc = bacc.Bacc("TRN2", target_bir_lowering=False, debug=False)

    I16 = mybir.dt.int16
    flow = nc.dram_tensor("flow", [BPC, 2, HF, WF], F32, kind="ExternalInput")
    mask = nc.dram_tensor("mask", [BPC, 1, HF, WF], F32, kind="ExternalInput")
    giF0 = nc.dram_tensor("giF0", [128, 16], I16, kind="ExternalInput")
    giF1 = nc.dram_tensor("giF1", [128, 16], I16, kind="ExternalInput")
    giM = nc.dram_tensor("giM", [128, 16], I16, kind="ExternalInput")
    WYF = nc.dram_tensor("WYF", [128, 256], F32, kind="ExternalInput")
    WYM = nc.dram_tensor("WYM", [64, 256], F32, kind="ExternalInput")
    WX2 = nc.dram_tensor("WX2", [128, 128], F32, kind="ExternalInput")
    SXYD = nc.dram_tensor("SXYD", [128, 128], F32, kind="ExternalInput")
    GRIDC = nc.dram_tensor("GRIDC", [128, 128], F32, kind="ExternalInput")
    C6 = nc.dram_tensor("C6", [128, 48], F32, kind="ExternalInput")
    EQF = nc.dram_tensor("EQF", [6, 288], F32, kind="ExternalInput")
    IDN64 = nc.dram_tensor("IDN64", [64, 64], F32, kind="ExternalInput")
    Hout = nc.dram_tensor("H", [BPC, 3, 3], F32, kind="ExternalOutput")

    V = nc.vector
    A = nc.scalar
    T = nc.tensor
    S = nc.sync
    G = nc.gpsimd

    y0 = cc.y0
    x0 = cc.x0

    with tile.TileContext(nc) as tc:
        with (
            tc.tile_pool(name="sb", bufs=1) as pool,
            tc.tile_pool(name="ps", bufs=1, space="PSUM") as psp,
        ):
            # ---------------- data + constant DMAs ----------------
            # only columns [CL0, CL0+NCW) of each 576-wide row are sampled
            CL0 = int(x0[0])        # 28
            NCW = 536               # covers max segment view extent (564 - 28)
            tF = pool.tile([128, 4, 2, NCW], F32)  # [p=(kd,b,c)][s][rowpair g][col]
            tM = pool.tile([128, 4, NCW], F32)     # [p=(a,kd,b)][s][col]

            fl = flow.ap().rearrange("b c h w -> (b c h w)").unsqueeze(0)
            ml = mask.ap().rearrange("b c h w -> (b c h w)").unsqueeze(0)

            def cin(eng, name, src, shape):
                t = pool.tile(list(shape), F32, tag=name)
                eng.dma_start(t[tuple(slice(0, s_) for s_ in shape)], src[:])
                return t

            # gather index tables + early-needed constants first (small)
            giF0_t = pool.tile([128, 16], I16, tag="giF0")
            S.dma_start(giF0_t[:, :], giF0[:])
            giF1_t = pool.tile([128, 16], I16, tag="giF1")
            A.dma_start(giF1_t[:, :], giF1[:])
            giM_t = pool.tile([128, 16], I16, tag="giM")
            S.dma_start(giM_t[:, :], giM[:])
            WYF_t = cin(S, "WYF", WYF, (128, 256))
            WX2_t = cin(A, "WX2", WX2, (128, 128))
            SXYD_t = cin(A, "SXYD", SXYD, (128, 128))
            GRIDC_t = cin(A, "GRIDC", GRIDC, (128, 128))

            # flow rows: s0/s1 on the HWDGE queues, s2/s3 via the gpsimd
            # software DGE (stripes across the other DMA engines)
            for s in range(4):
                nkd = 8 if s < 3 else 7
                eng = (S, A, G, G)[s]
                for g in (0, 1):
                    eng.dma_start(
                        tF[0 : 16 * nkd, s, g, :],
                        _dview(fl, (y0[s] + g) * WF + CL0,
                               [[37 * WF, nkd], [HW, 16], [1, NCW]]),
                    )
            for g in (0, 1):  # last grid row (k=31) breaks the stride
                G.dma_start(
                    tF[112:128, 3, g, :],
                    _dview(fl, (y0[31] + g) * WF + CL0, [[HW, 16], [1, NCW]]),
                )
            # mask rows: per (s, a) one strided DMA, h0 slots first
            for s in range(4):
                nkd = 8 if s < 3 else 7
                for a in (0, 1):
                    eng = S if a == 0 else A
                    eng.dma_start(
                        tM[64 * a : 64 * a + 8 * nkd, s, :],
                        _dview(ml, (y0[s] + a) * WF + CL0,
                               [[37 * WF, nkd], [HW, 8], [1, NCW]]),
                    )
            for a in (0, 1):
                eng = S if a == 0 else A
                eng.dma_start(
                    tM[64 * a + 56 : 64 * a + 64, 3, :],
                    _dview(ml, (y0[31] + a) * WF + CL0, [[HW, 8], [1, NCW]]),
                )

            # late-needed constants on the software DGE after the flow
            WYM_t = cin(G, "WYM", WYM, (64, 256))
            IDN64_t = cin(G, "IDN64", IDN64, (64, 64))
            C6_t = cin(G, "C6", C6, (128, 48))
            EQF_t = cin(G, "EQF", EQF, (6, 288))

            # ---------------- memset constants ----------------
            ONESC = pool.tile([128, 1], F32, tag="ONESC")
            V.memset(ONESC[:, :], 1.0 / NPTS)
            ONESR = pool.tile([1, 128], F32, tag="ONESR")
            V.memset(ONESR[:, :], 1.0)
            IEYE = pool.tile([8, 9], F32, tag="IEYE")
            V.memset(IEYE[:, :], 0.0)
            V.memset(IEYE[:, 0:9:4], 1.0)
            EPSD = pool.tile([8, 8], F32, tag="EPSD")
            V.memset(EPSD[:, 0:2], float(cc.epsd[0]))
            V.memset(EPSD[:, 3:5], float(cc.epsd[3]))
            V.memset(EPSD[:, 2:6:3], float(cc.epsd[2]))
            V.memset(EPSD[:, 6:8], float(cc.epsd[6]))
            HN = pool.tile([8, 9], F32)
            V.memset(HN[:, 8:9], 1.0)

            # ---------------- column select (gpsimd gathers) -----------------
            # GxF[p][s][g][i][c2]; per-half gathers overlap the later DMAs
            GxF = pool.tile([128, 4, 2, 32, 2], F32)
            GxM = pool.tile([128, 4, 32, 2], F32)
            for h, gi_t in ((0, giF0_t), (1, giF1_t)):
                G.ap_gather(
                    out_ap=GxF[:, 2 * h : 2 * h + 2, :, :, :],
                    in_ap=tF[:, :, :, :].rearrange("p s g w -> p (s g w)"),
                    idxs_ap=gi_t[:, :],
                    channels=128,
                    num_elems=4 * 2 * NCW,
                    d=1,
                    num_idxs=256,
                )
            G.ap_gather(
                out_ap=GxM[:, :, :, :],
                in_ap=tM[:, :, :].rearrange("p s w -> p (s w)"),
                idxs_ap=giM_t[:, :],
                channels=128,
                num_elems=4 * NCW,
                d=1,
                num_idxs=256,
            )

            # ---------------- flow bilinear ----------------
            dF = pool.tile([128, 256], F32)
            VF = pool.tile([128, 256], F32)
            dxF = pool.tile([128, 128], F32)
            sampF = pool.tile([128, 128], F32)
            psF = psp.tile([128, 128], F32)
            psSum = psp.tile([1, 128], F32, tag="pss")
            PQs = pool.tile([128, 128], F32)  # [pl=(s,i)][t 8][b 8][c 2]
            for h in (0, 1):
                sl = slice(2 * h, 2 * h + 2)
                hf = slice(128 * h, 128 * h + 128)
                g0 = GxF[:, sl, 0, :, :]
                g1 = GxF[:, sl, 1, :, :]
                dh = dF[:, hf]
                dh4 = dh.rearrange("p (s i c) -> p s i c", s=2, i=32, c=2)
                VF4 = VF[:, hf].rearrange("p (s i c) -> p s i c", s=2, i=32, c=2)
                V.tensor_tensor(out=dh4, in0=g1, in1=g0, op=ALU.subtract)
                V.tensor_tensor(out=dh, in0=dh, in1=WYF_t[:, hf], op=ALU.mult)
                V.tensor_tensor(out=VF4, in0=dh4, in1=g0, op=ALU.add)
                VFv = VF[:, hf].rearrange("p (s i c) -> p s i c", s=2, i=32, c=2)
                hx = slice(64 * h, 64 * h + 64)
                dxv = dxF[:, hx].rearrange("p (s i) -> p s i", s=2)
                V.tensor_tensor(out=dxv, in0=VFv[:, :, :, 1], in1=VFv[:, :, :, 0],
                                op=ALU.subtract)
                V.tensor_tensor(out=dxF[:, hx], in0=dxF[:, hx], in1=WX2_t[:, hx],
                                op=ALU.mult)
                V.tensor_tensor(out=sampF[:, hx].rearrange("p (s i) -> p s i", s=2),
                                in0=dxv, in1=VFv[:, :, :, 0], op=ALU.add)
                # transpose-with-scale for this half; mean accumulates on PE
                T.matmul(psF[hx, :], sampF[:, hx], SXYD_t[:, :],
                         start=True, stop=True)
                V.tensor_tensor(out=PQs[hx, :], in0=psF[hx, :],
                                in1=GRIDC_t[hx, :], op=ALU.add)
                T.matmul(psSum[:, :], ONESC[hx, :], PQs[hx, :],
                         start=(h == 0), stop=(h == 1))

            # ---------------- mask bilinear ----------------
            Gm = GxM[:, :, :, :].rearrange("p s i c -> p (s i c)")
            HIc = pool.tile([64, 256], F32)
            V.tensor_copy(HIc[:, :], Gm[64:128, :])
            dM = pool.tile([64, 256], F32)
            VM = pool.tile([64, 256], F32)
            V.tensor_tensor(out=dM[:, :], in0=HIc[:, :], in1=Gm[0:64, :],
                            op=ALU.subtract)
            V.tensor_tensor(out=dM[:, :], in0=dM[:, :], in1=WYM_t[:, :], op=ALU.mult)
            V.tensor_tensor(out=VM[:, :], in0=dM[:, :], in1=Gm[0:64, :], op=ALU.add)
            VMv = VM[:, :].rearrange("p (s i c) -> p s i c", s=4, i=32, c=2)
            dxM = pool.tile([64, 128], F32)
            sampM = pool.tile([64, 128], F32)
            dmv = dxM[:, :].rearrange("p (s i) -> p s i", s=4)
            V.tensor_tensor(out=dmv, in0=VMv[:, :, :, 1], in1=VMv[:, :, :, 0],
                            op=ALU.subtract)
            V.tensor_tensor(out=dxM[:, :], in0=dxM[:, :], in1=WX2_t[0:64, :],
                            op=ALU.mult)
            V.tensor_tensor(out=sampM[:, :].rearrange("p (s i) -> p s i", s=4),
                            in0=dmv, in1=VMv[:, :, :, 0], op=ALU.add)
            psM0 = psp.tile([128, 64], F32, tag="psm0")
            T.transpose(psM0[:, :], sampM[:, :], IDN64_t[:, :])

            # ---------------- mean over points ----------------
            MRow = pool.tile([1, 16], F32)   # [b 8][c 2] means
            V.tensor_reduce(
                out=MRow[:, :],
                in_=psSum[:, :].rearrange("o (t q) -> o q t", t=8, q=16),
                axis=mybir.AxisListType.X,
                op=ALU.add,
            )
            psMB = psp.tile([128, 16], F32, tag="psb")
            T.matmul(psMB[:, :], ONESR[:, :], MRow[:, :], start=True, stop=True)

            # ---------------- centered features ----------------
            CXY = pool.tile([128, 128], F32)
            V.tensor_tensor(
                out=CXY[:, :].rearrange("p (t q) -> p t q", t=8, q=16),
                in0=PQs[:, :].rearrange("p (t q) -> p t q", t=8, q=16),
                in1=psMB[:, :].unsqueeze(1).broadcast_to([128, 8, 16]),
                op=ALU.subtract,
            )
            SQ = pool.tile([128, 128], F32)
            V.tensor_tensor(out=SQ[:, :], in0=CXY[:, :], in1=CXY[:, :], op=ALU.mult)
            R2 = pool.tile([128, 64], F32)
            SQv = SQ[:, :].rearrange("p (t b c) -> p t b c", t=8, b=8, c=2)
            R2v = R2[:, :].rearrange("p (t b) -> p t b", t=8)
            V.tensor_tensor(out=R2v, in0=SQv[:, :, :, 0], in1=SQv[:, :, :, 1], op=ALU.add)

            # ---------------- D = [w, w*cx, w*cy, w*r2] ----------------
            D = pool.tile([128, 256], F32)    # [pl][t 8][q 4][b 8]
            Dv = D[:, :].rearrange("p (t q b) -> p q t b", q=4, b=8)
            V.tensor_scalar(out=Dv[:, 0, :, :],
                            in0=psM0[:, :].rearrange("p (t b) -> p t b", t=8),
                            scalar1=0.0, op0=ALU.max, scalar2=None)
            d12 = D[:, :].rearrange("p (t q b) -> p t q b", q=4, b=8)[:, :, 1:3, :]
            cxy12 = CXY[:, :].rearrange("p (t b c) -> p t c b", t=8, b=8, c=2)
            wb2 = Dv[:, 0, :, :].unsqueeze(2).broadcast_to([128, 8, 2, 8])
            V.tensor_tensor(out=d12, in0=cxy12, in1=wb2, op=ALU.mult)
            V.tensor_tensor(out=Dv[:, 3, :, :], in0=R2v, in1=Dv[:, 0, :, :], op=ALU.mult)

            # ---------------- moments: 8 accumulating matmuls ----------------
            psMom = psp.tile([6, 32], F32)
            for t in range(8):
                T.matmul(psMom[:, :], C6_t[:, 6 * t : 6 * t + 6],
                         D[:, 32 * t : 32 * t + 32],
                         start=(t == 0), stop=(t == 7))
            M6 = pool.tile([6, 32], F32)
            V.tensor_copy(M6[:, :], psMom[:, :])

            # ---------------- per-batch scalars to partitions --------------
            psPS = psp.tile([8, 24], F32, tag="psc")
            T.transpose(psPS[:, 0:1], MRow[:, 0:16:2], IDN64_t[0:1, 0:1])
            T.transpose(psPS[:, 8:9], MRow[:, 1:16:2], IDN64_t[0:1, 0:1])
            T.transpose(psPS[:, 16:17], M6[0:1, 0:8], IDN64_t[0:1, 0:1])
            MXs = psPS[:, 0:1]
            MYs = psPS[:, 8:9]
            GT = pool.tile([8, 1], F32)
            V.tensor_scalar(out=GT[:, :], in0=psPS[:, 16:17],
                            scalar1=NPTS * 1e-4, op0=ALU.is_gt, scalar2=None)

            # ---------------- assemble [A^T A | A^T b] ----------------
            psAUG = psp.tile([8, 72], F32, tag="psa")
            for q in range(4):
                T.matmul(psAUG[:, :], M6[:, 8 * q : 8 * q + 8],
                         EQF_t[:, 72 * q : 72 * q + 72],
                         start=(q == 0), stop=(q == 3))
            AUG = pool.tile([8, 72], F32)
            V.tensor_copy(AUG[:, :], psAUG[:, :])
            V.tensor_tensor(out=AUG[:, 0:71:10], in0=AUG[:, 0:71:10],
                            in1=EPSD[:, :], op=ALU.add)

            # ---------------- Gauss-Jordan (in place) ----------------
            PIV = pool.tile([8, 1], F32)
            U = pool.tile([8, 8, 9], F32)
            AUGv = AUG[:, :].rearrange("p (r c) -> p r c", r=8)
            for k in range(8):
                w_ = 9 - k
                V.reciprocal(PIV[:, :], AUG[:, 10 * k : 10 * k + 1])
                rowk = AUG[:, 10 * k : 9 * k + 9]
                V.tensor_tensor(out=rowk, in0=rowk,
                                in1=PIV[:, :].broadcast_to([8, w_]), op=ALU.mult)
                fcol = AUG[:, k : 72 : 9]
                V.tensor_tensor(
                    out=U[:, :, 0:w_],
                    in0=fcol.unsqueeze(2).broadcast_to([8, 8, w_]),
                    in1=rowk.unsqueeze(1).broadcast_to([8, 8, w_]),
                    op=ALU.mult,
                )
                if k > 0:
                    V.tensor_tensor(out=AUGv[:, 0:k, k:9], in0=AUGv[:, 0:k, k:9],
                                    in1=U[:, 0:k, 0:w_], op=ALU.subtract)
                if k < 7:
                    V.tensor_tensor(out=AUGv[:, k + 1 : 8, k:9],
                                    in0=AUGv[:, k + 1 : 8, k:9],
                                    in1=U[:, k + 1 : 8, 0:w_], op=ALU.subtract)

            # ---------------- denormalize + gate ----------------
            V.tensor_copy(HN[:, 0:8], AUG[:, 8:72:9])
            H2 = pool.tile([8, 9], F32)
            V.scalar_tensor_tensor(out=H2[:, 0:3], in0=HN[:, 6:9], scalar=MXs,
                                   in1=HN[:, 0:3], op0=ALU.mult, op1=ALU.add)
            V.scalar_tensor_tensor(out=H2[:, 3:6], in0=HN[:, 6:9], scalar=MYs,
                                   in1=HN[:, 3:6], op0=ALU.mult, op1=ALU.add)
            V.tensor_copy(H2[:, 6:9], HN[:, 6:9])
            T1 = pool.tile([8, 3], F32)
            V.tensor_scalar(out=T1[:, :], in0=H2[:, 0:9:3], scalar1=cc.ca,
                            op0=ALU.mult, scalar2=None)
            V.scalar_tensor_tensor(out=T1[:, :], in0=H2[:, 1:9:3], scalar=cc.da,
                                   in1=T1[:, :], op0=ALU.mult, op1=ALU.add)
            V.tensor_tensor(out=H2[:, 2:9:3], in0=T1[:, :], in1=H2[:, 2:9:3], op=ALU.add)
            RECD = pool.tile([8, 1], F32)
            V.reciprocal(RECD[:, :], H2[:, 8:9])
            V.tensor_scalar(out=H2[:, :], in0=H2[:, :], scalar1=RECD[:, :],
                            op0=ALU.mult, scalar2=None)
            IG = pool.tile([8, 1], F32)
            TI = pool.tile([8, 9], F32)
            OUTt = pool.tile([8, 9], F32)
            V.tensor_scalar(out=IG[:, :], in0=GT[:, :], scalar1=-1.0, op0=ALU.mult,
                            scalar2=1.0, op1=ALU.add)
            V.tensor_scalar(out=TI[:, :], in0=IEYE[:, :], scalar1=IG[:, :],
                            op0=ALU.mult, scalar2=None)
            V.scalar_tensor_tensor(out=OUTt[:, :], in0=H2[:, :], scalar=GT[:, :],
                                   in1=TI[:, :], op0=ALU.mult, op1=ALU.add)
            S.dma_start(Hout.ap().rearrange("b r c -> b (r c)"), OUTt[:, :])

    nc.compile()
    return nc


# ---------------------------------------------------------------------------
# host wrapper
# ---------------------------------------------------------------------------

_CACHE = {}


def _get(img_h, img_w):
    key = (int(img_h), int(img_w))
    if key not in _CACHE:
        cc = _Consts(*key)
        _CACHE[key] = (cc, _build_program(cc))
    return _CACHE[key]


def _in_maps(cc, flow, mask):
    flow = np.ascontiguousarray(flow, np.float32)
    mask = np.ascontiguousarray(mask, np.float32)
    maps = []
    for c in range(NCORES):
        maps.append({
            "flow": flow[c * BPC : (c + 1) * BPC],
            "mask": mask[c * BPC : (c + 1) * BPC],
            "giF0": cc.giF[0], "giF1": cc.giF[1], "giM": cc.giM,
            "WYF": cc.WYF, "WYM": cc.WYM, "WX2": cc.WX2,
            "SXYD": cc.SXYD, "GRIDC": cc.GRIDC, "C6": cc.C6,
            "EQF": cc.EQF, "IDN64": cc.IDN64,
        })
    return maps


def run(flow, mask, img_h, img_w, trace=False, **spmd_kwargs):
    cc, nc = _get(img_h, img_w)
    res = bass_utils.run_bass_kernel_spmd(
        nc, _in_maps(cc, flow, mask), list(range(NCORES)), trace=trace, **spmd_kwargs
    )
    out = np.concatenate([res.results[c]["H"] for c in range(NCORES)], axis=0)
    return out.astype(np.float32), res


def kernel(flow, mask, img_h, img_w):
    out, _ = run(flow, mask, img_h, img_w)
    return out


# revision 45
# speedup vs baseline: 1.4936x; 1.4936x over previous
"""Trainium2 Bass kernel for DifferentiableDLT (batched weighted-DLT homography fit).

Contract: kernel(**inputs) takes FULL inputs
    flow (64, 2, 320, 576) f32, mask (64, 1, 320, 576) f32, img_h, img_w
and returns the FULL output (64, 3, 3) f32.

Strategy (pure data parallel, 8 batches/core x 8 cores):
  The 1024 sample points form a fixed separable 32x32 grid, so bilinear
  sampling touches only 64 of 320 rows (32 pairs of adjacent rows) and 64 of
  576 columns.  All row/column indices are compile-time constants, so per
  core we:
    1. Fetch the needed rows with STATIC strided HW DMAs (no gpsimd gather):
       every 4th grid row is a uniform 37-row stride in HBM, so 5 flow DMAs
       (pair-rows of both channels of all 8 batches per s-slot) and 10 mask
       DMAs cover everything.
    2. Select the 64 needed columns with 9 uniform-stride-run copies per
       tensor (DVE), merging both bilinear column taps into each copy.
    3. Bilinear lerp in y then x (DVE tensor ops with constant weight tiles).
    4. One PE transpose-with-scale puts points on partitions; a constant
       grid-offset tile is added during the PSUM drain.
    5. Hartley-normalize dst with a COMPILE-TIME scale sigma0 (any scale
       choice changes the result only through the 1e-6 regularizer) and a
       runtime mean; build weighted features D = [w, w*cx, w*cy, w*r2]; all
       24+ moments come from one wide PE matmul C6^T @ D plus a block-fold.
    6. Assemble the 8x9 augmented normal equations directly per batch
       (batch on partitions) via 4 accumulating PE matmuls against constant
       E-matrices with sigma0/T_src column scaling folded in; solve with
       unpivoted in-place Gauss-Jordan.
    7. Denormalize in the pre-scaled solution space (runtime mean only),
       sign/scale fix, support gate, DMA out (8,3,3).
"""

import dataclasses
import math
import numpy as np

import concourse.bass as bass
import concourse.bacc as bacc
import concourse.mybir as mybir
from concourse import tile
from concourse import bass_utils

F32 = mybir.dt.float32
ALU = mybir.AluOpType

NCORES = 8
BPC = 8          # batches per core
HF, WF = 320, 576
HW = HF * WF
NG = 32          # grid is NG x NG points
NPTS = NG * NG
EPS = 1e-6

# ---------------------------------------------------------------------------
# host-side constant computation
# ---------------------------------------------------------------------------


def _grid_1d(size, n):
    m = int(size * 0.05)
    return np.linspace(m, size - m - 1, n)


def _segments(x0):
    """Maximal uniform-step segments (start, len, step) covering x0."""
    segs = []
    i = 0
    n = len(x0)
    while i < n:
        if i == n - 1:
            segs.append((i, 1, 1))
            break
        st = x0[i + 1] - x0[i]
        j = i + 1
        while j + 1 < n and x0[j + 1] - x0[j] == st:
            j += 1
        segs.append((i, j - i + 1, int(st)))
        i = j + 1
    return segs


class _Consts:
    def __init__(self, img_h, img_w):
        ys = _grid_1d(HF, NG)
        xs = _grid_1d(WF, NG)
        y0 = np.floor(ys).astype(np.int64)
        x0 = np.floor(xs).astype(np.int64)
        wy = (ys - y0).astype(np.float32)
        wx = (xs - x0).astype(np.float32)
        self.x0 = x0
        self.y0 = y0
        self.segs = _segments(x0)
        sx = np.float32((img_w - 1) / max(WF - 1, 1))
        sy = np.float32((img_h - 1) / max(HF - 1, 1))
        self.sx, self.sy = float(sx), float(sy)

        # constant Hartley normalization of source points (image coords)
        j = np.arange(NPTS) // NG
        i = np.arange(NPTS) % NG
        gx = xs[i] * np.float64(sx)
        gy = ys[j] * np.float64(sy)
        mx, my = gx.mean(), gy.mean()
        cxs, cys = gx - mx, gy - my
        s_src = max(np.sqrt(cxs * cxs + cys * cys).mean() / math.sqrt(2.0), 1e-8)
        u = cxs / s_src
        v = cys / s_src
        a_ts = 1.0 / s_src
        c_ts = -mx / s_src
        d_ts = -my / s_src
        self.ca = float(np.float32(c_ts / a_ts))
        self.da = float(np.float32(d_ts / a_ts))
        SIG0 = s_src  # compile-time dst normalization scale

        # ---- lerp weight tiles ----
        p = np.arange(128)
        sfree = np.arange(256) // 64
        self.WYF = wy[(p // 16)[:, None] * 4 + sfree[None, :]].astype(np.float32)
        pm = np.arange(64)
        self.WYM = wy[(pm // 8)[:, None] * 4 + sfree[None, :]].astype(np.float32)
        i128 = np.arange(128) % 32
        self.WX2 = np.tile(wx[i128][None, :], (128, 1)).astype(np.float32)

        # ---- transpose-scale diag + grid offsets ----
        # sampF partition p = kd*16 + b*2 + c ; psF free j = same indexing
        jj = np.arange(128)
        c_j = jj % 2
        kd_j = jj // 16
        sxy = np.where(c_j == 0, np.float64(sx), np.float64(sy))
        self.SXYD = (np.eye(128) * sxy[None, :]).astype(np.float32)
        # GRIDC[pl=(s,i)][j=(kd,b,c)]
        pl = np.arange(128)
        s_pl = pl // 32
        i_pl = pl % 32
        gxv = xs * np.float64(sx)
        gyv = ys * np.float64(sy)
        G = np.where(
            c_j[None, :] == 0,
            gxv[i_pl][:, None],
            gyv[(kd_j[None, :] * 4 + s_pl[:, None])],
        )
        self.GRIDC = G.astype(np.float32)

        # ---- point-feature matrix C6 (128, 8*6): F=[1, uu, uv, u, vv, v] ----
        # ("ones" first so the sum-of-weights moment lands on partition 0)
        # device point order n' = t*128 + pl, pl = s*32+i, grid row k = t*4+s
        PERM = [5, 0, 1, 2, 3, 4]  # new m-index -> old feature index
        feats = np.stack([u * u, u * v, u, v * v, v, np.ones_like(u)], -1)[:, PERM]
        tt = np.arange(NPTS) // 128
        plv = np.arange(NPTS) % 128
        orig = (tt * 4 + plv // 32) * 32 + plv % 32
        self.C6 = np.ascontiguousarray(
            feats[orig].reshape(8, 128, 6).transpose(1, 0, 2).reshape(128, 48)
        ).astype(np.float32)

        # ---- E matrices: AUG[b][r*9+c] = sum_q sum_m M6[m,(q,b)] * EQF[m, 72q + r*9+c]
        E = np.zeros((4, 6, 72))
        sym = [[0, 1, 2], [1, 3, 4], [2, 4, 5]]
        for r in range(3):
            for c in range(3):
                m = sym[r][c]
                E[0, m, r * 9 + c] += 1
                E[0, m, (r + 3) * 9 + (c + 3)] += 1
        cr = [[0, 1], [1, 3], [2, 4]]
        for q, r0 in ((1, 0), (2, 3)):
            for r in range(3):
                for c2 in range(2):
                    m = cr[r][c2]
                    E[q, m, (r0 + r) * 9 + 6 + c2] += -1
                    E[q, m, (6 + c2) * 9 + (r0 + r)] += -1
            for r, m in ((0, 2), (1, 4), (2, 5)):
                E[q, m, (r0 + r) * 9 + 8] += 1
        rb = [[0, 1], [1, 3]]
        for r in range(2):
            for c2 in range(2):
                E[3, rb[r][c2], (6 + r) * 9 + 6 + c2] += 1
        E[3, 2, 6 * 9 + 8] += -1
        E[3, 4, 7 * 9 + 8] += -1
        E = E[:, PERM, :]  # match the C6 feature reorder
        # fold sigma0 moment normalization
        inv0 = 1.0 / SIG0
        E[1] *= inv0
        E[2] *= inv0
        E[3] *= inv0 * inv0
        # fold solution-space column scaling (y = S^-1 h)
        Scc = np.ones(9)
        Scc[[0, 1, 3, 4]] = 1.0 / (a_ts * SIG0)
        Scc[[2, 5]] = 1.0 / SIG0
        Scc[[6, 7]] = 1.0 / a_ts
        colidx = np.arange(72) % 9
        E = E * Scc[colidx][None, None, :]
        self.EQF = np.ascontiguousarray(
            E.transpose(1, 0, 2).reshape(6, 288)
        ).astype(np.float32)
        self.epsd = (EPS * Scc[:8]).astype(np.float32)  # diag regularizer values

        self.IDN64 = np.eye(64, dtype=np.float32)


# ---------------------------------------------------------------------------
# device program
# ---------------------------------------------------------------------------


def _dview(ap, offset, dims):
    """Custom strided DRAM view (strides/offset in elements)."""
    return dataclasses.replace(ap, offset=int(offset), ap=[list(d) for d in dims])


def _build_program(cc: _Consts):
    nc = bacc.Bacc("TRN2", target_bir_lowering=False, debug=False)

    flow = nc.dram_tensor("flow", [BPC, 2, HF, WF], F32, kind="ExternalInput")
    mask = nc.dram_tensor("mask", [BPC, 1, HF, WF], F32, kind="ExternalInput")
    WYF = nc.dram_tensor("WYF", [128, 256], F32, kind="ExternalInput")
    WYM = nc.dram_tensor("WYM", [64, 256], F32, kind="ExternalInput")
    WX2 = nc.dram_tensor("WX2", [128, 128], F32, kind="ExternalInput")
    SXYD = nc.dram_tensor("SXYD", [128, 128], F32, kind="ExternalInput")
    GRIDC = nc.dram_tensor("GRIDC", [128, 128], F32, kind="ExternalInput")
    C6 = nc.dram_tensor("C6", [128, 48], F32, kind="ExternalInput")
    EQF = nc.dram_tensor("EQF", [6, 288], F32, kind="ExternalInput")
    IDN64 = nc.dram_tensor("IDN64", [64, 64], F32, kind="ExternalInput")
    Hout = nc.dram_tensor("H", [BPC, 3, 3], F32, kind="ExternalOutput")

    V = nc.vector
    A = nc.scalar
    T = nc.tensor
    S = nc.sync
    G = nc.gpsimd

    y0 = cc.y0
    x0 = cc.x0

    with tile.TileContext(nc) as tc:
        with (
            tc.tile_pool(name="sb", bufs=1) as pool,
            tc.tile_pool(name="ps", bufs=1, space="PSUM") as psp,
        ):
            # ---------------- data + constant DMAs ----------------
            # only columns [CL0, CL0+NCW) of each 576-wide row are sampled
            CL0 = int(x0[0])        # 28
            NCW = 536               # covers max segment view extent (564 - 28)
            tF = pool.tile([128, 4, 2, NCW], F32)  # [p=(kd,b,c)][s][rowpair g][col]
            tM = pool.tile([128, 4, NCW], F32)     # [p=(a,kd,b)][s][col]

            fl = flow.ap().rearrange("b c h w -> (b c h w)").unsqueeze(0)
            ml = mask.ap().rearrange("b c h w -> (b c h w)").unsqueeze(0)

            def cin(eng, name, src, shape):
                t = pool.tile(list(shape), F32, tag=name)
                eng.dma_start(t[tuple(slice(0, s_) for s_ in shape)], src[:])
                return t

            # early-needed constants first on the HWDGE queues (small)
            WYF_t = cin(S, "WYF", WYF, (128, 256))
            WX2_t = cin(A, "WX2", WX2, (128, 128))
            SXYD_t = cin(A, "SXYD", SXYD, (128, 128))
            GRIDC_t = cin(A, "GRIDC", GRIDC, (128, 128))

            # flow rows: s0/s1 on the HWDGE queues, s2/s3 via the gpsimd
            # software DGE (stripes across the other DMA engines)
            for s in range(4):
                nkd = 8 if s < 3 else 7
                eng = (S, A, G, G)[s]
                for g in (0, 1):
                    eng.dma_start(
                        tF[0 : 16 * nkd, s, g, :],
                        _dview(fl, (y0[s] + g) * WF + CL0,
                               [[37 * WF, nkd], [HW, 16], [1, NCW]]),
                    )
            for g in (0, 1):  # last grid row (k=31) breaks the stride
                G.dma_start(
                    tF[112:128, 3, g, :],
                    _dview(fl, (y0[31] + g) * WF + CL0, [[HW, 16], [1, NCW]]),
                )
            # mask rows: per (s, a) one strided DMA, h0 slots first
            for s in range(4):
                nkd = 8 if s < 3 else 7
                for a in (0, 1):
                    eng = S if a == 0 else A
                    eng.dma_start(
                        tM[64 * a : 64 * a + 8 * nkd, s, :],
                        _dview(ml, (y0[s] + a) * WF + CL0,
                               [[37 * WF, nkd], [HW, 8], [1, NCW]]),
                    )
            for a in (0, 1):
                eng = S if a == 0 else A
                eng.dma_start(
                    tM[64 * a + 56 : 64 * a + 64, 3, :],
                    _dview(ml, (y0[31] + a) * WF + CL0, [[HW, 8], [1, NCW]]),
                )

            # late-needed constants on the software DGE after the flow
            WYM_t = cin(G, "WYM", WYM, (64, 256))
            IDN64_t = cin(G, "IDN64", IDN64, (64, 64))
            C6_t = cin(G, "C6", C6, (128, 48))
            EQF_t = cin(G, "EQF", EQF, (6, 288))

            # ---------------- memset constants ----------------
            ONESC = pool.tile([128, 1], F32, tag="ONESC")
            V.memset(ONESC[:, :], 1.0 / NPTS)
            ONESR = pool.tile([1, 128], F32, tag="ONESR")
            V.memset(ONESR[:, :], 1.0)
            IEYE = pool.tile([8, 9], F32, tag="IEYE")
            V.memset(IEYE[:, :], 0.0)
            V.memset(IEYE[:, 0:9:4], 1.0)
            EPSD = pool.tile([8, 8], F32, tag="EPSD")
            V.memset(EPSD[:, 0:2], float(cc.epsd[0]))
            V.memset(EPSD[:, 3:5], float(cc.epsd[3]))
            V.memset(EPSD[:, 2:6:3], float(cc.epsd[2]))
            V.memset(EPSD[:, 6:8], float(cc.epsd[6]))
            HN = pool.tile([8, 9], F32)
            V.memset(HN[:, 8:9], 1.0)

            # ---------------- column select ----------------
            # flow on DVE (h-halves overlap the later DMAs); mask on the
            # otherwise-idle Scalar engine
            GxF = pool.tile([128, 4, 2, 32, 2], F32)  # [s][g][i][c2]
            GxM = pool.tile([128, 4, 32, 2], F32)
            for h in (0, 1):
                sl = slice(2 * h, 2 * h + 2)
                for g in (0, 1):
                    for (i0, L, st) in cc.segs:
                        base = int(x0[i0]) - CL0
                        src = tF[:, sl, g, base : base + L * st].rearrange(
                            "p s (i r) -> p s i r", r=st
                        )[:, :, :, 0:2]
                        V.tensor_copy(GxF[:, sl, g, i0 : i0 + L, :], src)
            for (i0, L, st) in cc.segs:
                base = int(x0[i0]) - CL0
                src = tM[:, :, base : base + L * st].rearrange(
                    "p s (i r) -> p s i r", r=st
                )[:, :, :, 0:2]
                A.copy(GxM[:, :, i0 : i0 + L, :], src)

            # ---------------- flow bilinear ----------------
            dF = pool.tile([128, 256], F32)
            VF = pool.tile([128, 256], F32)
            dxF = pool.tile([128, 128], F32)
            sampF = pool.tile([128, 128], F32)
            psF = psp.tile([128, 128], F32)
            psSum = psp.tile([1, 128], F32, tag="pss")
            PQs = pool.tile([128, 128], F32)  # [pl=(s,i)][t 8][b 8][c 2]
            for h in (0, 1):
                sl = slice(2 * h, 2 * h + 2)
                hf = slice(128 * h, 128 * h + 128)
                g0 = GxF[:, sl, 0, :, :]
                g1 = GxF[:, sl, 1, :, :]
                dh = dF[:, hf]
                dh4 = dh.rearrange("p (s i c) -> p s i c", s=2, i=32, c=2)
                VF4 = VF[:, hf].rearrange("p (s i c) -> p s i c", s=2, i=32, c=2)
                V.tensor_tensor(out=dh4, in0=g1, in1=g0, op=ALU.subtract)
                V.tensor_tensor(out=dh, in0=dh, in1=WYF_t[:, hf], op=ALU.mult)
                V.tensor_tensor(out=VF4, in0=dh4, in1=g0, op=ALU.add)
                VFv = VF[:, hf].rearrange("p (s i c) -> p s i c", s=2, i=32, c=2)
                hx = slice(64 * h, 64 * h + 64)
                dxv = dxF[:, hx].rearrange("p (s i) -> p s i", s=2)
                V.tensor_tensor(out=dxv, in0=VFv[:, :, :, 1], in1=VFv[:, :, :, 0],
                                op=ALU.subtract)
                V.tensor_tensor(out=dxF[:, hx], in0=dxF[:, hx], in1=WX2_t[:, hx],
                                op=ALU.mult)
                V.tensor_tensor(out=sampF[:, hx].rearrange("p (s i) -> p s i", s=2),
                                in0=dxv, in1=VFv[:, :, :, 0], op=ALU.add)
                # transpose-with-scale for this half; mean accumulates on PE
                T.matmul(psF[hx, :], sampF[:, hx], SXYD_t[:, :],
                         start=True, stop=True)
                V.tensor_tensor(out=PQs[hx, :], in0=psF[hx, :],
                                in1=GRIDC_t[hx, :], op=ALU.add)
                T.matmul(psSum[:, :], ONESC[hx, :], PQs[hx, :],
                         start=(h == 0), stop=(h == 1))

            # ---------------- mask bilinear ----------------
            Gm = GxM[:, :, :, :].rearrange("p s i c -> p (s i c)")
            HIc = pool.tile([64, 256], F32)
            A.copy(HIc[:, :], Gm[64:128, :])
            dM = pool.tile([64, 256], F32)
            VM = pool.tile([64, 256], F32)
            V.tensor_tensor(out=dM[:, :], in0=HIc[:, :], in1=Gm[0:64, :],
                            op=ALU.subtract)
            V.tensor_tensor(out=dM[:, :], in0=dM[:, :], in1=WYM_t[:, :], op=ALU.mult)
            V.tensor_tensor(out=VM[:, :], in0=dM[:, :], in1=Gm[0:64, :], op=ALU.add)
            VMv = VM[:, :].rearrange("p (s i c) -> p s i c", s=4, i=32, c=2)
            dxM = pool.tile([64, 128], F32)
            sampM = pool.tile([64, 128], F32)
            dmv = dxM[:, :].rearrange("p (s i) -> p s i", s=4)
            V.tensor_tensor(out=dmv, in0=VMv[:, :, :, 1], in1=VMv[:, :, :, 0],
                            op=ALU.subtract)
            V.tensor_tensor(out=dxM[:, :], in0=dxM[:, :], in1=WX2_t[0:64, :],
                            op=ALU.mult)
            V.tensor_tensor(out=sampM[:, :].rearrange("p (s i) -> p s i", s=4),
                            in0=dmv, in1=VMv[:, :, :, 0], op=ALU.add)
            psM0 = psp.tile([128, 64], F32, tag="psm0")
            T.transpose(psM0[:, :], sampM[:, :], IDN64_t[:, :])

            # ---------------- mean over points ----------------
            MRow = pool.tile([1, 16], F32)   # [b 8][c 2] means
            V.tensor_reduce(
                out=MRow[:, :],
                in_=psSum[:, :].rearrange("o (t q) -> o q t", t=8, q=16),
                axis=mybir.AxisListType.X,
                op=ALU.add,
            )
            psMB = psp.tile([128, 16], F32, tag="psb")
            T.matmul(psMB[:, :], ONESR[:, :], MRow[:, :], start=True, stop=True)

            # ---------------- centered features ----------------
            CXY = pool.tile([128, 128], F32)
            V.tensor_tensor(
                out=CXY[:, :].rearrange("p (t q) -> p t q", t=8, q=16),
                in0=PQs[:, :].rearrange("p (t q) -> p t q", t=8, q=16),
                in1=psMB[:, :].unsqueeze(1).broadcast_to([128, 8, 16]),
                op=ALU.subtract,
            )
            SQ = pool.tile([128, 128], F32)
            V.tensor_tensor(out=SQ[:, :], in0=CXY[:, :], in1=CXY[:, :], op=ALU.mult)
            R2 = pool.tile([128, 64], F32)
            SQv = SQ[:, :].rearrange("p (t b c) -> p t b c", t=8, b=8, c=2)
            R2v = R2[:, :].rearrange("p (t b) -> p t b", t=8)
            V.tensor_tensor(out=R2v, in0=SQv[:, :, :, 0], in1=SQv[:, :, :, 1], op=ALU.add)

            # ---------------- D = [w, w*cx, w*cy, w*r2] ----------------
            D = pool.tile([128, 256], F32)    # [pl][t 8][q 4][b 8]
            Dv = D[:, :].rearrange("p (t q b) -> p q t b", q=4, b=8)
            V.tensor_scalar(out=Dv[:, 0, :, :],
                            in0=psM0[:, :].rearrange("p (t b) -> p t b", t=8),
                            scalar1=0.0, op0=ALU.max, scalar2=None)
            d12 = D[:, :].rearrange("p (t q b) -> p t q b", q=4, b=8)[:, :, 1:3, :]
            cxy12 = CXY[:, :].rearrange("p (t b c) -> p t c b", t=8, b=8, c=2)
            wb2 = Dv[:, 0, :, :].unsqueeze(2).broadcast_to([128, 8, 2, 8])
            V.tensor_tensor(out=d12, in0=cxy12, in1=wb2, op=ALU.mult)
            V.tensor_tensor(out=Dv[:, 3, :, :], in0=R2v, in1=Dv[:, 0, :, :], op=ALU.mult)

            # ---------------- moments: 8 accumulating matmuls ----------------
            psMom = psp.tile([6, 32], F32)
            for t in range(8):
                T.matmul(psMom[:, :], C6_t[:, 6 * t : 6 * t + 6],
                         D[:, 32 * t : 32 * t + 32],
                         start=(t == 0), stop=(t == 7))
            M6 = pool.tile([6, 32], F32)
            V.tensor_copy(M6[:, :], psMom[:, :])

            # ---------------- per-batch scalars to partitions --------------
            psPS = psp.tile([8, 24], F32, tag="psc")
            T.transpose(psPS[:, 0:1], MRow[:, 0:16:2], IDN64_t[0:1, 0:1])
            T.transpose(psPS[:, 8:9], MRow[:, 1:16:2], IDN64_t[0:1, 0:1])
            T.transpose(psPS[:, 16:17], M6[0:1, 0:8], IDN64_t[0:1, 0:1])
            MXs = psPS[:, 0:1]
            MYs = psPS[:, 8:9]
            GT = pool.tile([8, 1], F32)
            V.tensor_scalar(out=GT[:, :], in0=psPS[:, 16:17],
                            scalar1=NPTS * 1e-4, op0=ALU.is_gt, scalar2=None)

            # ---------------- assemble [A^T A | A^T b] ----------------
            psAUG = psp.tile([8, 72], F32, tag="psa")
            for q in range(4):
                T.matmul(psAUG[:, :], M6[:, 8 * q : 8 * q + 8],
                         EQF_t[:, 72 * q : 72 * q + 72],
                         start=(q == 0), stop=(q == 3))
            AUG = pool.tile([8, 72], F32)
            V.tensor_copy(AUG[:, :], psAUG[:, :])
            V.tensor_tensor(out=AUG[:, 0:71:10], in0=AUG[:, 0:71:10],
                            in1=EPSD[:, :], op=ALU.add)

            # ---------------- Gauss-Jordan (in place) ----------------
            PIV = pool.tile([8, 1], F32)
            U = pool.tile([8, 8, 9], F32)
            AUGv = AUG[:, :].rearrange("p (r c) -> p r c", r=8)
            for k in range(8):
                w_ = 9 - k
                V.reciprocal(PIV[:, :], AUG[:, 10 * k : 10 * k + 1])
                rowk = AUG[:, 10 * k : 9 * k + 9]
                V.tensor_tensor(out=rowk, in0=rowk,
                                in1=PIV[:, :].broadcast_to([8, w_]), op=ALU.mult)
                fcol = AUG[:, k : 72 : 9]
                V.tensor_tensor(
                    out=U[:, :, 0:w_],
                    in0=fcol.unsqueeze(2).broadcast_to([8, 8, w_]),
                    in1=rowk.unsqueeze(1).broadcast_to([8, 8, w_]),
                    op=ALU.mult,
                )
                if k > 0:
                    V.tensor_tensor(out=AUGv[:, 0:k, k:9], in0=AUGv[:, 0:k, k:9],
                                    in1=U[:, 0:k, 0:w_], op=ALU.subtract)
                if k < 7:
                    V.tensor_tensor(out=AUGv[:, k + 1 : 8, k:9],
                                    in0=AUGv[:, k + 1 : 8, k:9],
                                    in1=U[:, k + 1 : 8, 0:w_], op=ALU.subtract)

            # ---------------- denormalize + gate ----------------
            V.tensor_copy(HN[:, 0:8], AUG[:, 8:72:9])
            H2 = pool.tile([8, 9], F32)
            V.scalar_tensor_tensor(out=H2[:, 0:3], in0=HN[:, 6:9], scalar=MXs,
                                   in1=HN[:, 0:3], op0=ALU.mult, op1=ALU.add)
            V.scalar_tensor_tensor(out=H2[:, 3:6], in0=HN[:, 6:9], scalar=MYs,
                                   in1=HN[:, 3:6], op0=ALU.mult, op1=ALU.add)
            V.tensor_copy(H2[:, 6:9], HN[:, 6:9])
            T1 = pool.tile([8, 3], F32)
            V.tensor_scalar(out=T1[:, :], in0=H2[:, 0:9:3], scalar1=cc.ca,
                            op0=ALU.mult, scalar2=None)
            V.scalar_tensor_tensor(out=T1[:, :], in0=H2[:, 1:9:3], scalar=cc.da,
                                   in1=T1[:, :], op0=ALU.mult, op1=ALU.add)
            V.tensor_tensor(out=H2[:, 2:9:3], in0=T1[:, :], in1=H2[:, 2:9:3], op=ALU.add)
            RECD = pool.tile([8, 1], F32)
            V.reciprocal(RECD[:, :], H2[:, 8:9])
            V.tensor_scalar(out=H2[:, :], in0=H2[:, :], scalar1=RECD[:, :],
                            op0=ALU.mult, scalar2=None)
            IG = pool.tile([8, 1], F32)
            TI = pool.tile([8, 9], F32)
            OUTt = pool.tile([8, 9], F32)
            V.tensor_scalar(out=IG[:, :], in0=GT[:, :], scalar1=-1.0, op0=ALU.mult,
                            scalar2=1.0, op1=ALU.add)
            V.tensor_scalar(out=TI[:, :], in0=IEYE[:, :], scalar1=IG[:, :],
                            op0=ALU.mult, scalar2=None)
            V.scalar_tensor_tensor(out=OUTt[:, :], in0=H2[:, :], scalar=GT[:, :],
                                   in1=TI[:, :], op0=ALU.mult, op1=ALU.add)
            S.dma_start(Hout.ap().rearrange("b r c -> b (r c)"), OUTt[:, :])

    nc.compile()
    return nc


# ---------------------------------------------------------------------------
# host wrapper
# ---------------------------------------------------------------------------

_CACHE = {}


def _get(img_h, img_w):
    key = (int(img_h), int(img_w))
    if key not in _CACHE:
        cc = _Consts(*key)
        _CACHE[key] = (cc, _build_program(cc))
    return _CACHE[key]


def _in_maps(cc, flow, mask):
    flow = np.ascontiguousarray(flow, np.float32)
    mask = np.ascontiguousarray(mask, np.float32)
    maps = []
    for c in range(NCORES):
        maps.append({
            "flow": flow[c * BPC : (c + 1) * BPC],
            "mask": mask[c * BPC : (c + 1) * BPC],
            "WYF": cc.WYF, "WYM": cc.WYM, "WX2": cc.WX2,
            "SXYD": cc.SXYD, "GRIDC": cc.GRIDC, "C6": cc.C6,
            "EQF": cc.EQF, "IDN64": cc.IDN64,
        })
    return maps


def run(flow, mask, img_h, img_w, trace=False, **spmd_kwargs):
    cc, nc = _get(img_h, img_w)
    res = bass_utils.run_bass_kernel_spmd(
        nc, _in_maps(cc, flow, mask), list(range(NCORES)), trace=trace, **spmd_kwargs
    )
    out = np.concatenate([res.results[c]["H"] for c in range(NCORES)], axis=0)
    return out.astype(np.float32), res


def kernel(flow, mask, img_h, img_w):
    out, _ = run(flow, mask, img_h, img_w)
    return out
